# revision 53
# baseline (speedup 1.0000x reference)
"""Trainium2 Bass kernel for nn_LocalRNN (local GRU, chunked scan).

Problem: B=32, S=2048, I=H=256, ksize=16. Each ksize-chunk runs a GRU from
h0=0, so the 32*128=4096 chunks are independent length-16 GRU chains.

Sharding: data-parallel over chunks — core c gets batch rows [4c:4c+4],
i.e. 512 chains. Weights replicated.

Per-core layout ("transposed"): gate/hidden dim on partitions, chain (seq)
index on the free dim. Two seq groups (G=2 x NS=256) ping-pong so one
group's elementwise chain hides under the other group's matmuls.

Per step t and group g (PSUM banks r, z, n, h — 4 per group, 8 total):

  gates[3H, seqs] = W_ih @ x_t^T + W_hh @ h_{t-1}^T     (PSUM accumulation)
  bank_h is seeded with b_hn by a tiny K=1 "bias matmul" (lhsT = bias row,
  moving = ones) before the W_hn h accumulation, so tmp below is a single
  full-bank DVE op instead of two per-half stt ops.
  r = sigmoid(psum_r + (b_ih+b_hh)_r)                    (ScalarE, bias port)
  z = sigmoid(psum_z + (b_ih+b_hh)_z)
  tmp = psum_hn * r; pren = (psum_in + b_in) + tmp; n = tanh(pren)
  h = n + z*(h_prev - n)

Startup: ~46 junk matmuls warm the PE HAM clock gate (1.2 -> 2.4 GHz)
while the weight/x DMAs land (r/z weight columns are DMA'd first since they
gate the first matmuls); x is DMA'd in 4-step 512KB blocks so descriptor
generation doesn't stall the PE mid-kernel; outputs are staged in SBUF and
written back in 4-step blocks (per-step for the final block, so the last
DMA isn't serialized behind the closing chain).

PE emission order per step: both groups' x-side matmuls first (independent
of the recurrence, interleaved across groups so each LDWEIGHTS feeds two
matmuls), then per group the h-side matmuls with the r-gate first (its
sigmoid leads the elementwise chain). On the final step the two groups'
h-matmuls interleave so both closing chains launch back-to-back.

Matmul operands and SBUF elementwise tensors are fp16 (values are O(1) so
fp16 range is safe); PSUM accumulation is fp32. Host pre-transposes x /
weights into DMA-friendly contiguous blocks and inverts the output layout
at the end.
"""

import sys

for _p in ("/opt/trn_rl_repo", "/root/.axon_site"):
    if _p not in sys.path:
        sys.path.insert(0, _p)

import ml_dtypes  # noqa: F401
import numpy as np

import concourse.bass as bass  # noqa: F401
import concourse.tile as tile
from concourse import bacc, mybir
from concourse.bass_utils import run_bass_kernel_spmd

# Problem constants (hardcoded per harness contract).
B, S, I, H = 32, 2048, 256, 256
KSIZE = 16
NCORES = 8
ROWS_PER_CORE = B // NCORES            # 4 batch rows per core
CHUNKS_PER_ROW = S // KSIZE            # 128
SEQS = ROWS_PER_CORE * CHUNKS_PER_ROW  # 512 chains per core
G = 2                                  # seq groups per core
NS = SEQS // G                         # 256 seqs per group
KT = 2                                 # contraction tiles (I/128 = H/128 = 2)
TBLK = 4                               # steps per DMA block
NBLK = KSIZE // TBLK                   # 4 blocks

WARM_MMS = 46                          # junk matmuls to warm the PE clock

F32 = mybir.dt.float32
F16 = mybir.dt.float16
AF = mybir.ActivationFunctionType
OP = mybir.AluOpType

MM_DT = F16
NP_MM_DT = np.float16


def build_nc():
    nc = bacc.Bacc("TRN2", target_bir_lowering=False, debug=False)

    # Inputs (host pre-transposed, contiguous per-DMA blocks).
    # xt[g, b, p, tt, k, s] = x_shard[seq=g*NS+s, t=b*TBLK+tt, i=k*128+p]
    xt_d = nc.dram_tensor(
        "xt", [G, NBLK, 128, TBLK, KT, NS], MM_DT, kind="ExternalInput"
    )
    # wih_t[p, k, m] = W_ih[m, k*128+p]  (transposed weight, lhsT layout)
    wih_d = nc.dram_tensor("wih_t", [128, KT, 3 * H], MM_DT, kind="ExternalInput")
    whh_d = nc.dram_tensor("whh_t", [128, KT, 3 * H], MM_DT, kind="ExternalInput")
    # brz[p, mi] = (b_ih+b_hh)[mi*128+p] for mi in 0..3 (r0,r1,z0,z1)
    brz_d = nc.dram_tensor("brz", [128, 4], F32, kind="ExternalInput")
    # bin[p, m] = b_ih[2H + m*128 + p]  (stt scalar port for pren)
    bin_d = nc.dram_tensor("bin", [128, 2], F32, kind="ExternalInput")
    # bias-matmul lhsT rows: [b_hn0, b_hn1], each 128 wide.
    bias2_d = nc.dram_tensor("bias2", [1, 256], MM_DT, kind="ExternalInput")
    # out[g, b, p, tt, m, s] = h_t[seq=g*NS+s, t=b*TBLK+tt, hdim=m*128+p]
    out_d = nc.dram_tensor(
        "out", [G, NBLK, 128, TBLK, 2, NS], MM_DT, kind="ExternalOutput"
    )

    with tile.TileContext(nc) as tc:
        with (
            tc.tile_pool(name="consts", bufs=1) as consts,
            tc.tile_pool(name="xp", bufs=4) as xp,
            tc.tile_pool(name="ps", bufs=2, space="PSUM") as ps,
            tc.tile_pool(name="work", bufs=4) as work,
            tc.tile_pool(name="ho", bufs=4) as hop,
        ):
            # --- PE warm-up: junk matmuls while DMAs land (HAM 1.2->2.4GHz).
            junk = consts.tile([128, 128], MM_DT)
            # GPSIMD memset: its preamble is shorter than Vector's, so the
            # warm-up matmuls (which read junk) can start earlier.
            nc.gpsimd.memset(junk[:], 0.0)
            ones = consts.tile([1, NS], MM_DT)
            nc.vector.memset(ones[:], 1.0)
            warm_ps = ps.tile([128, 2, NS], F32, tag="r")
            for _ in range(WARM_MMS):
                nc.tensor.matmul(
                    warm_ps[:, 0, :128], junk[:], junk[:],
                    start=True, stop=True, skip_group_check=True,
                )

            # --- Constants. DMA order puts first-needed data first.
            wih = consts.tile([128, KT, 3 * H], MM_DT)
            # r weight columns first — they gate the first real matmuls —
            # then the t=0 x slices, then z/n columns and the rest.
            nc.sync.dma_start(wih[:, :, :H], wih_d.ap()[:, :, :H])
            bias2 = consts.tile([1, 256], MM_DT)
            nc.sync.dma_start(bias2[:], bias2_d.ap())
            xblk = {}
            for g in range(G):
                xb = xp.tile([128, TBLK, KT, NS], MM_DT, tag="x")
                # Scalar queue: parallel descriptor-gen with the weight DMAs.
                nc.scalar.dma_start(xb[:, :1], xt_d.ap()[g, 0, :, :1])
                xblk[g] = xb
            nc.sync.dma_start(wih[:, :, H : 2 * H], wih_d.ap()[:, :, H : 2 * H])
            brz = consts.tile([128, 4], F32)
            nc.scalar.dma_start(brz[:], brz_d.ap())
            bin_ = consts.tile([128, 2], F32)
            nc.sync.dma_start(bin_[:], bin_d.ap())
            nc.sync.dma_start(wih[:, :, 2 * H :], wih_d.ap()[:, :, 2 * H :])
            for g in range(G):
                nc.sync.dma_start(xblk[g][:, 1:], xt_d.ap()[g, 0, :, 1:])
            whh = consts.tile([128, KT, 3 * H], MM_DT)
            nc.sync.dma_start(whh[:], whh_d.ap())

            h_prev = [None] * G   # AP view of previous step's h
            ho_t = [None] * G     # current output staging tile
            gates = [None] * G    # (r_t, z_t) SBUF fp16 tiles
            n_t = [None] * G

            def h_mm(bank, g, mi, m, stop, start=False):
                col = slice(mi * 128, (mi + 1) * 128)
                for k in range(KT):
                    nc.tensor.matmul(
                        bank[:, m, :], whh[:, k, col], h_prev[g][:, k, :],
                        start=(start and k == 0), stop=(stop and k == KT - 1),
                    )

            for t in range(KSIZE):
                blk, tt = divmod(t, TBLK)
                if tt == 0 and blk + 1 < NBLK:
                    # Prefetch next x block for both groups.
                    for g in range(G):
                        xb = xp.tile([128, TBLK, KT, NS], MM_DT, tag="x")
                        nc.sync.dma_start(xb[:], xt_d.ap()[g, blk + 1])
                        xblk[(g, blk + 1)] = xb
                if tt == 0 and blk > 0:
                    for g in range(G):
                        xblk[g] = xblk.pop((g, blk))

                bank_r, bank_z, bank_n, bank_h = {}, {}, {}, {}
                # PART A: x-side matmuls — independent of h'(t-1); both
                # groups interleaved per weight tile (shared LDWEIGHTS).
                # One start=True per (bank, half) accumulation group; the
                # group's stop rides the last h-side matmul (t>0).
                for g in range(G):
                    bank_r[g] = ps.tile([128, 2, NS], F32, tag="r", name="bank_r")
                    bank_z[g] = ps.tile([128, 2, NS], F32, tag="z", name="bank_z")
                    bank_n[g] = ps.tile([128, 2, NS], F32, tag="n", name="bank_n")
                    bank_h[g] = ps.tile([128, 2, NS], F32, tag="h", name="bank_h")
                def x_mms(bank, base, start, x_stop):
                    for m in range(2):
                        for k in range(KT):
                            for g in range(G):
                                nc.tensor.matmul(
                                    bank[g][:, m, :],
                                    wih[:, k, base + m * 128 : base + (m + 1) * 128],
                                    xblk[g][:, tt, k, :],
                                    start=(start and m == 0 and k == 0),
                                    stop=(x_stop and m == 1 and k == KT - 1),
                                )

                def b_mms(bank, base, stop):
                    for g in range(G):
                        for m in range(2):
                            nc.tensor.matmul(
                                bank[g][:, m, :],
                                bias2[:, base + m * 128 : base + (m + 1) * 128],
                                ones[:], start=(m == 0), stop=(stop and m == 1),
                            )

                b_mms(bank_h, 0, stop=(t == 0))
                x_mms(bank_r, 0, start=True, x_stop=(t == 0))
                x_mms(bank_z, 256, start=True, x_stop=(t == 0))
                x_mms(bank_n, 512, start=True, x_stop=True)

                # PART B: h-side matmuls + sigmoids (r first — it leads).
                # On the final step the two groups' h-matmuls interleave so
                # both closing chains launch back-to-back.
                if t == KSIZE - 1:
                    for m in range(2):
                        for g in range(G):
                            h_mm(bank_r[g], g, m, m, stop=(m == 1))
                    for m in range(2):
                        for g in range(G):
                            h_mm(bank_h[g], g, 4 + m, m, stop=(m == 1))
                    for m in range(2):
                        for g in range(G):
                            h_mm(bank_z[g], g, 2 + m, m, stop=(m == 1))
                for g in range(G):
                    if 0 < t < KSIZE - 1:
                        for m in range(2):
                            h_mm(bank_r[g], g, m, m, stop=(m == 1))
                        for m in range(2):
                            h_mm(bank_h[g], g, 4 + m, m, stop=(m == 1))
                        for m in range(2):
                            h_mm(bank_z[g], g, 2 + m, m, stop=(m == 1))
                    r_t = work.tile([128, 2, NS], MM_DT, tag="rg")
                    z_t = work.tile([128, 2, NS], MM_DT, tag="zg")
                    for mi in range(2):
                        nc.scalar.activation(
                            r_t[:, mi, :], bank_r[g][:, mi, :], AF.Sigmoid,
                            bias=brz[:, mi : mi + 1],
                        )
                    for mi in range(2):
                        nc.scalar.activation(
                            z_t[:, mi, :], bank_z[g][:, mi, :], AF.Sigmoid,
                            bias=brz[:, 2 + mi : 3 + mi],
                        )
                    gates[g] = (r_t, z_t)

                # PART C: n-path. tmp = (psum_hn + b_hn)*r;
                # pren = (psum_in + b_in) + tmp; n = tanh(pren).
                for g in range(G):
                    r_t, z_t = gates[g]
                    tmp = work.tile([128, 2, NS], MM_DT, tag="tmp")
                    pren = work.tile([128, 2, NS], MM_DT, tag="pren")
                    nt = work.tile([128, 2, NS], MM_DT, tag="n")
                    # tmp = psum_hn * r  (b_hn pre-added by the bias matmul)
                    nc.vector.tensor_tensor(
                        tmp[:], bank_h[g][:], r_t[:], op=OP.mult
                    )
                    for m in range(2):
                        nc.vector.scalar_tensor_tensor(
                            pren[:, m, :], bank_n[g][:, m, :],
                            bin_[:, m : m + 1], tmp[:, m, :],
                            op0=OP.add, op1=OP.add,
                        )
                    nc.scalar.activation(nt[:], pren[:], AF.Tanh)
                    n_t[g] = nt

                # PART D: h-update + staged output DMA.
                for g in range(G):
                    _, z_t = gates[g]
                    if tt == 0:
                        ho_t[g] = hop.tile(
                            [128, TBLK, 2, NS], MM_DT, tag="ho", name="ho"
                        )
                    hnew = ho_t[g][:, tt, :, :]
                    e = work.tile([128, 2, NS], MM_DT, tag="e")
                    if t == 0:
                        # h1 = n - z*n
                        nc.vector.tensor_tensor(
                            e[:], z_t[:], n_t[g][:], op=OP.mult
                        )
                        nc.vector.tensor_tensor(
                            hnew, n_t[g][:], e[:], op=OP.subtract
                        )
                    else:
                        d = work.tile([128, 2, NS], MM_DT, tag="d")
                        # h = n + z*(h_prev - n)
                        nc.vector.tensor_tensor(
                            d[:], h_prev[g][:], n_t[g][:], op=OP.subtract
                        )
                        nc.vector.tensor_tensor(e[:], z_t[:], d[:], op=OP.mult)
                        nc.vector.tensor_tensor(
                            hnew, e[:], n_t[g][:], op=OP.add
                        )
                    h_prev[g] = hnew
                    if blk == NBLK - 1:
                        # Write each step of the last block out directly so
                        # the final DMA isn't serialized after the chain.
                        # The very last step goes out on the Scalar queue,
                        # which is idle once its final tanh retires.
                        eng = nc.scalar if t == KSIZE - 1 else nc.sync
                        eng.dma_start(
                            out_d.ap()[g, blk, :, tt], ho_t[g][:, tt, :, :]
                        )
                    elif tt == TBLK - 1:
                        nc.sync.dma_start(out_d.ap()[g, blk], ho_t[g][:])

    nc.compile()
    return nc


_NC_CACHE = None


def _get_nc():
    global _NC_CACHE
    if _NC_CACHE is None:
        _NC_CACHE = build_nc()
    return _NC_CACHE


def _prep_shared(W_ih, W_hh, b_ih, b_hh):
    wih_t = np.ascontiguousarray(
        W_ih.T.reshape(KT, 128, 3 * H).transpose(1, 0, 2)
    ).astype(NP_MM_DT)
    whh_t = np.ascontiguousarray(
        W_hh.T.reshape(KT, 128, 3 * H).transpose(1, 0, 2)
    ).astype(NP_MM_DT)
    bsum = b_ih + b_hh
    brz = np.ascontiguousarray(bsum[: 2 * H].reshape(4, 128).T)
    bias2 = np.ascontiguousarray(
        b_hh[2 * H :].reshape(1, 256)
    ).astype(NP_MM_DT)
    bin_ = np.ascontiguousarray(b_ih[2 * H :].reshape(2, 128).T)
    return wih_t, whh_t, brz, bias2, bin_


def _prep_core_inputs(x, shared, core):
    wih_t, whh_t, brz, bias2, bin_ = shared
    xc = x[core * ROWS_PER_CORE : (core + 1) * ROWS_PER_CORE]  # [4, S, I]
    xc = xc.reshape(SEQS, KSIZE, I)
    # xt[g, b, p, tt, k, s] = xc[g*NS+s, b*TBLK+tt, k*128+p]
    xt = np.ascontiguousarray(
        xc.reshape(G, NS, NBLK, TBLK, KT, 128).transpose(0, 2, 5, 3, 4, 1)
    ).astype(NP_MM_DT)
    return {
        "xt": xt,
        "wih_t": wih_t,
        "whh_t": whh_t,
        "brz": brz,
        "bias2": bias2,
        "bin": bin_,
    }


def kernel(x, W_ih, W_hh, b_ih, b_hh, ksize):
    x = np.asarray(x, dtype=np.float32)
    W_ih = np.asarray(W_ih, dtype=np.float32)
    W_hh = np.asarray(W_hh, dtype=np.float32)
    b_ih = np.asarray(b_ih, dtype=np.float32)
    b_hh = np.asarray(b_hh, dtype=np.float32)
    assert int(ksize) == KSIZE and x.shape == (B, S, I)

    shared = _prep_shared(W_ih, W_hh, b_ih, b_hh)
    in_maps = [_prep_core_inputs(x, shared, c) for c in range(NCORES)]
    nc = _get_nc()
    res = run_bass_kernel_spmd(nc, in_maps, core_ids=list(range(NCORES)))

    out = np.empty((B, S, H), dtype=np.float32)
    for c in range(NCORES):
        oc = np.asarray(res.results[c]["out"]).astype(np.float32)
        # oc[g, b, p, tt, m, s] -> h[seq=g*NS+s, t=b*TBLK+tt, hdim=m*128+p]
        hc = oc.transpose(0, 5, 1, 3, 4, 2).reshape(SEQS, KSIZE, H)
        out[c * ROWS_PER_CORE : (c + 1) * ROWS_PER_CORE] = hc.reshape(
            ROWS_PER_CORE, S, H
        )
    return out


# revision 54
# speedup vs baseline: 1.0062x; 1.0062x over previous
"""Trainium2 Bass kernel for nn_LocalRNN (local GRU, chunked scan).

Problem: B=32, S=2048, I=H=256, ksize=16. Each ksize-chunk runs a GRU from
h0=0, so the 32*128=4096 chunks are independent length-16 GRU chains.

Sharding: data-parallel over chunks — core c gets batch rows [4c:4c+4],
i.e. 512 chains. Weights replicated.

Per-core layout ("transposed"): gate/hidden dim on partitions, chain (seq)
index on the free dim. Two seq groups (G=2 x NS=256) ping-pong so one
group's elementwise chain hides under the other group's matmuls.

Per step t and group g (PSUM banks r, z, n, h — 4 per group, 8 total):

  gates[3H, seqs] = W_ih @ x_t^T + W_hh @ h_{t-1}^T     (PSUM accumulation)
  bank_h is seeded with b_hn by a tiny K=1 "bias matmul" (lhsT = bias row,
  moving = ones) before the W_hn h accumulation, so tmp below is a single
  full-bank DVE op instead of two per-half stt ops.
  r = sigmoid(psum_r + (b_ih+b_hh)_r)                    (ScalarE, bias port)
  z = sigmoid(psum_z + (b_ih+b_hh)_z)
  tmp = psum_hn * r; pren = (psum_in + b_in) + tmp; n = tanh(pren)
  h = n + z*(h_prev - n)

Startup: ~46 junk matmuls warm the PE HAM clock gate (1.2 -> 2.4 GHz)
while the weight/x DMAs land (r/z weight columns are DMA'd first since they
gate the first matmuls); x is DMA'd in 4-step 512KB blocks so descriptor
generation doesn't stall the PE mid-kernel; outputs are staged in SBUF and
written back in 4-step blocks (per-step for the final block, so the last
DMA isn't serialized behind the closing chain).

PE emission order per step: both groups' x-side matmuls first (independent
of the recurrence, interleaved across groups so each LDWEIGHTS feeds two
matmuls), then per group the h-side matmuls with the r-gate first (its
sigmoid leads the elementwise chain). On the final step the two groups'
h-matmuls interleave so both closing chains launch back-to-back.

Matmul operands and SBUF elementwise tensors are fp16 (values are O(1) so
fp16 range is safe); PSUM accumulation is fp32. Host pre-transposes x /
weights into DMA-friendly contiguous blocks and inverts the output layout
at the end.
"""

import sys

for _p in ("/opt/trn_rl_repo", "/root/.axon_site"):
    if _p not in sys.path:
        sys.path.insert(0, _p)

import ml_dtypes  # noqa: F401
import numpy as np

import concourse.bass as bass  # noqa: F401
import concourse.tile as tile
from concourse import bacc, mybir
from concourse.bass_utils import run_bass_kernel_spmd

# Problem constants (hardcoded per harness contract).
B, S, I, H = 32, 2048, 256, 256
KSIZE = 16
NCORES = 8
ROWS_PER_CORE = B // NCORES            # 4 batch rows per core
CHUNKS_PER_ROW = S // KSIZE            # 128
SEQS = ROWS_PER_CORE * CHUNKS_PER_ROW  # 512 chains per core
G = 2                                  # seq groups per core
NS = SEQS // G                         # 256 seqs per group
KT = 2                                 # contraction tiles (I/128 = H/128 = 2)
TBLK = 4                               # steps per DMA block
NBLK = KSIZE // TBLK                   # 4 blocks

WARM_MMS = 46                          # junk matmuls to warm the PE clock

F32 = mybir.dt.float32
F16 = mybir.dt.float16
AF = mybir.ActivationFunctionType
OP = mybir.AluOpType

MM_DT = F16
NP_MM_DT = np.float16


def build_nc():
    nc = bacc.Bacc("TRN2", target_bir_lowering=False, debug=False)

    # Inputs (host pre-transposed, contiguous per-DMA blocks).
    # xt[g, b, p, tt, k, s] = x_shard[seq=g*NS+s, t=b*TBLK+tt, i=k*128+p]
    xt_d = nc.dram_tensor(
        "xt", [G, NBLK, 128, TBLK, KT, NS], MM_DT, kind="ExternalInput"
    )
    # Per-gate contiguous weight tensors (lhsT layout): w?[p, k, m128] —
    # column-slices of a combined tensor DMA as 512B strided descriptors
    # and land ~2us later at startup.
    wr_d = nc.dram_tensor("wr", [128, KT, H], MM_DT, kind="ExternalInput")
    wz_d = nc.dram_tensor("wz", [128, KT, H], MM_DT, kind="ExternalInput")
    wn_d = nc.dram_tensor("wn", [128, KT, H], MM_DT, kind="ExternalInput")
    whh_d = nc.dram_tensor("whh_t", [128, KT, 3 * H], MM_DT, kind="ExternalInput")
    # brz[p, mi] = (b_ih+b_hh)[mi*128+p] for mi in 0..3 (r0,r1,z0,z1)
    brz_d = nc.dram_tensor("brz", [128, 4], F32, kind="ExternalInput")
    # bin[p, m] = b_ih[2H + m*128 + p]  (stt scalar port for pren)
    bin_d = nc.dram_tensor("bin", [128, 2], F32, kind="ExternalInput")
    # bias-matmul lhsT rows: [b_hn0, b_hn1], each 128 wide.
    bias2_d = nc.dram_tensor("bias2", [1, 256], MM_DT, kind="ExternalInput")
    # out[g, b, p, tt, m, s] = h_t[seq=g*NS+s, t=b*TBLK+tt, hdim=m*128+p]
    out_d = nc.dram_tensor(
        "out", [G, NBLK, 128, TBLK, 2, NS], MM_DT, kind="ExternalOutput"
    )

    with tile.TileContext(nc) as tc:
        with (
            tc.tile_pool(name="consts", bufs=1) as consts,
            tc.tile_pool(name="xp", bufs=4) as xp,
            tc.tile_pool(name="ps", bufs=2, space="PSUM") as ps,
            tc.tile_pool(name="work", bufs=4) as work,
            tc.tile_pool(name="ho", bufs=4) as hop,
        ):
            # --- PE warm-up: junk matmuls while DMAs land (HAM 1.2->2.4GHz).
            junk = consts.tile([128, 128], MM_DT)
            # GPSIMD memset: its preamble is shorter than Vector's, so the
            # warm-up matmuls (which read junk) can start earlier.
            nc.gpsimd.memset(junk[:], 0.0)
            ones = consts.tile([1, NS], MM_DT)
            nc.vector.memset(ones[:], 1.0)
            warm_ps = ps.tile([128, 2, NS], F32, tag="r")
            for _ in range(WARM_MMS):
                nc.tensor.matmul(
                    warm_ps[:, 0, :128], junk[:], junk[:],
                    start=True, stop=True, skip_group_check=True,
                )

            # --- Constants. DMA order puts first-needed data first.
            # r-gate weights first (contiguous) — they gate the first
            # matmuls — then the t=0 x slices, then z/n and the rest.
            wr = consts.tile([128, KT, H], MM_DT)
            nc.sync.dma_start(wr[:], wr_d.ap())
            bias2 = consts.tile([1, 256], MM_DT)
            nc.sync.dma_start(bias2[:], bias2_d.ap())
            xblk = {}
            for g in range(G):
                xb = xp.tile([128, TBLK, KT, NS], MM_DT, tag="x")
                # Scalar queue: parallel descriptor-gen with the weight DMAs.
                nc.scalar.dma_start(xb[:, :1], xt_d.ap()[g, 0, :, :1])
                xblk[g] = xb
            wz = consts.tile([128, KT, H], MM_DT)
            nc.sync.dma_start(wz[:], wz_d.ap())
            brz = consts.tile([128, 4], F32)
            nc.scalar.dma_start(brz[:], brz_d.ap())
            wn = consts.tile([128, KT, H], MM_DT)
            nc.sync.dma_start(wn[:], wn_d.ap())
            bin_ = consts.tile([128, 2], F32)
            nc.sync.dma_start(bin_[:], bin_d.ap())
            for g in range(G):
                nc.sync.dma_start(xblk[g][:, 1:], xt_d.ap()[g, 0, :, 1:])
            whh = consts.tile([128, KT, 3 * H], MM_DT)
            nc.sync.dma_start(whh[:], whh_d.ap())

            h_prev = [None] * G   # AP view of previous step's h
            ho_t = [None] * G     # current output staging tile
            gates = [None] * G    # (r_t, z_t) SBUF fp16 tiles
            n_t = [None] * G

            def h_mm(bank, g, mi, m, stop, start=False):
                col = slice(mi * 128, (mi + 1) * 128)
                for k in range(KT):
                    nc.tensor.matmul(
                        bank[:, m, :], whh[:, k, col], h_prev[g][:, k, :],
                        start=(start and k == 0), stop=(stop and k == KT - 1),
                    )

            for t in range(KSIZE):
                blk, tt = divmod(t, TBLK)
                if tt == 0 and blk + 1 < NBLK:
                    # Prefetch next x block for both groups.
                    for g in range(G):
                        xb = xp.tile([128, TBLK, KT, NS], MM_DT, tag="x")
                        nc.sync.dma_start(xb[:], xt_d.ap()[g, blk + 1])
                        xblk[(g, blk + 1)] = xb
                if tt == 0 and blk > 0:
                    for g in range(G):
                        xblk[g] = xblk.pop((g, blk))

                bank_r, bank_z, bank_n, bank_h = {}, {}, {}, {}
                # PART A: x-side matmuls — independent of h'(t-1); both
                # groups interleaved per weight tile (shared LDWEIGHTS).
                # One start=True per (bank, half) accumulation group; the
                # group's stop rides the last h-side matmul (t>0).
                for g in range(G):
                    bank_r[g] = ps.tile([128, 2, NS], F32, tag="r", name="bank_r")
                    bank_z[g] = ps.tile([128, 2, NS], F32, tag="z", name="bank_z")
                    bank_n[g] = ps.tile([128, 2, NS], F32, tag="n", name="bank_n")
                    bank_h[g] = ps.tile([128, 2, NS], F32, tag="h", name="bank_h")
                def x_mms(bank, wt, start, x_stop):
                    for m in range(2):
                        for k in range(KT):
                            for g in range(G):
                                nc.tensor.matmul(
                                    bank[g][:, m, :],
                                    wt[:, k, m * 128 : (m + 1) * 128],
                                    xblk[g][:, tt, k, :],
                                    start=(start and m == 0 and k == 0),
                                    stop=(x_stop and m == 1 and k == KT - 1),
                                )

                def b_mms(bank, base, stop):
                    for g in range(G):
                        for m in range(2):
                            nc.tensor.matmul(
                                bank[g][:, m, :],
                                bias2[:, base + m * 128 : base + (m + 1) * 128],
                                ones[:], start=(m == 0), stop=(stop and m == 1),
                            )

                b_mms(bank_h, 0, stop=(t == 0))
                x_mms(bank_r, wr, start=True, x_stop=(t == 0))
                x_mms(bank_z, wz, start=True, x_stop=(t == 0))
                x_mms(bank_n, wn, start=True, x_stop=True)

                # PART B: h-side matmuls + sigmoids (r first — it leads).
                # On the final step the two groups' h-matmuls interleave so
                # both closing chains launch back-to-back.
                if t == KSIZE - 1:
                    for m in range(2):
                        for g in range(G):
                            h_mm(bank_r[g], g, m, m, stop=(m == 1))
                    for m in range(2):
                        for g in range(G):
                            h_mm(bank_h[g], g, 4 + m, m, stop=(m == 1))
                    for m in range(2):
                        for g in range(G):
                            h_mm(bank_z[g], g, 2 + m, m, stop=(m == 1))
                for g in range(G):
                    if 0 < t < KSIZE - 1:
                        for m in range(2):
                            h_mm(bank_r[g], g, m, m, stop=(m == 1))
                        for m in range(2):
                            h_mm(bank_h[g], g, 4 + m, m, stop=(m == 1))
                        for m in range(2):
                            h_mm(bank_z[g], g, 2 + m, m, stop=(m == 1))
                    r_t = work.tile([128, 2, NS], MM_DT, tag="rg")
                    z_t = work.tile([128, 2, NS], MM_DT, tag="zg")
                    for mi in range(2):
                        nc.scalar.activation(
                            r_t[:, mi, :], bank_r[g][:, mi, :], AF.Sigmoid,
                            bias=brz[:, mi : mi + 1],
                        )
                    for mi in range(2):
                        nc.scalar.activation(
                            z_t[:, mi, :], bank_z[g][:, mi, :], AF.Sigmoid,
                            bias=brz[:, 2 + mi : 3 + mi],
                        )
                    gates[g] = (r_t, z_t)

                # PART C: n-path. tmp = (psum_hn + b_hn)*r;
                # pren = (psum_in + b_in) + tmp; n = tanh(pren).
                for g in range(G):
                    r_t, z_t = gates[g]
                    tmp = work.tile([128, 2, NS], MM_DT, tag="tmp")
                    pren = work.tile([128, 2, NS], MM_DT, tag="pren")
                    nt = work.tile([128, 2, NS], MM_DT, tag="n")
                    # tmp = psum_hn * r  (b_hn pre-added by the bias matmul)
                    nc.vector.tensor_tensor(
                        tmp[:], bank_h[g][:], r_t[:], op=OP.mult
                    )
                    for m in range(2):
                        nc.vector.scalar_tensor_tensor(
                            pren[:, m, :], bank_n[g][:, m, :],
                            bin_[:, m : m + 1], tmp[:, m, :],
                            op0=OP.add, op1=OP.add,
                        )
                    nc.scalar.activation(nt[:], pren[:], AF.Tanh)
                    n_t[g] = nt

                # PART D: h-update + staged output DMA.
                for g in range(G):
                    _, z_t = gates[g]
                    if tt == 0:
                        ho_t[g] = hop.tile(
                            [128, TBLK, 2, NS], MM_DT, tag="ho", name="ho"
                        )
                    hnew = ho_t[g][:, tt, :, :]
                    e = work.tile([128, 2, NS], MM_DT, tag="e")
                    if t == 0:
                        # h1 = n - z*n
                        nc.vector.tensor_tensor(
                            e[:], z_t[:], n_t[g][:], op=OP.mult
                        )
                        nc.vector.tensor_tensor(
                            hnew, n_t[g][:], e[:], op=OP.subtract
                        )
                    else:
                        d = work.tile([128, 2, NS], MM_DT, tag="d")
                        # h = n + z*(h_prev - n)
                        nc.vector.tensor_tensor(
                            d[:], h_prev[g][:], n_t[g][:], op=OP.subtract
                        )
                        nc.vector.tensor_tensor(e[:], z_t[:], d[:], op=OP.mult)
                        nc.vector.tensor_tensor(
                            hnew, e[:], n_t[g][:], op=OP.add
                        )
                    h_prev[g] = hnew
                    if blk == NBLK - 1:
                        # Write each step of the last block out directly so
                        # the final DMA isn't serialized after the chain.
                        # The very last step goes out on the Scalar queue,
                        # which is idle once its final tanh retires.
                        eng = nc.scalar if t == KSIZE - 1 else nc.sync
                        eng.dma_start(
                            out_d.ap()[g, blk, :, tt], ho_t[g][:, tt, :, :]
                        )
                    elif tt == TBLK - 1:
                        nc.sync.dma_start(out_d.ap()[g, blk], ho_t[g][:])

    nc.compile()
    return nc


_NC_CACHE = None


def _get_nc():
    global _NC_CACHE
    if _NC_CACHE is None:
        _NC_CACHE = build_nc()
    return _NC_CACHE


def _prep_shared(W_ih, W_hh, b_ih, b_hh):
    wih_t = W_ih.T.reshape(KT, 128, 3 * H).transpose(1, 0, 2)
    wr = np.ascontiguousarray(wih_t[:, :, :H]).astype(NP_MM_DT)
    wz = np.ascontiguousarray(wih_t[:, :, H : 2 * H]).astype(NP_MM_DT)
    wn = np.ascontiguousarray(wih_t[:, :, 2 * H :]).astype(NP_MM_DT)
    whh_t = np.ascontiguousarray(
        W_hh.T.reshape(KT, 128, 3 * H).transpose(1, 0, 2)
    ).astype(NP_MM_DT)
    bsum = b_ih + b_hh
    brz = np.ascontiguousarray(bsum[: 2 * H].reshape(4, 128).T)
    bias2 = np.ascontiguousarray(
        b_hh[2 * H :].reshape(1, 256)
    ).astype(NP_MM_DT)
    bin_ = np.ascontiguousarray(b_ih[2 * H :].reshape(2, 128).T)
    return wr, wz, wn, whh_t, brz, bias2, bin_


def _prep_core_inputs(x, shared, core):
    wr, wz, wn, whh_t, brz, bias2, bin_ = shared
    xc = x[core * ROWS_PER_CORE : (core + 1) * ROWS_PER_CORE]  # [4, S, I]
    xc = xc.reshape(SEQS, KSIZE, I)
    # xt[g, b, p, tt, k, s] = xc[g*NS+s, b*TBLK+tt, k*128+p]
    xt = np.ascontiguousarray(
        xc.reshape(G, NS, NBLK, TBLK, KT, 128).transpose(0, 2, 5, 3, 4, 1)
    ).astype(NP_MM_DT)
    return {
        "xt": xt,
        "wr": wr,
        "wz": wz,
        "wn": wn,
        "whh_t": whh_t,
        "brz": brz,
        "bias2": bias2,
        "bin": bin_,
    }


def kernel(x, W_ih, W_hh, b_ih, b_hh, ksize):
    x = np.asarray(x, dtype=np.float32)
    W_ih = np.asarray(W_ih, dtype=np.float32)
    W_hh = np.asarray(W_hh, dtype=np.float32)
    b_ih = np.asarray(b_ih, dtype=np.float32)
    b_hh = np.asarray(b_hh, dtype=np.float32)
    assert int(ksize) == KSIZE and x.shape == (B, S, I)

    shared = _prep_shared(W_ih, W_hh, b_ih, b_hh)
    in_maps = [_prep_core_inputs(x, shared, c) for c in range(NCORES)]
    nc = _get_nc()
    res = run_bass_kernel_spmd(nc, in_maps, core_ids=list(range(NCORES)))

    out = np.empty((B, S, H), dtype=np.float32)
    for c in range(NCORES):
        oc = np.asarray(res.results[c]["out"]).astype(np.float32)
        # oc[g, b, p, tt, m, s] -> h[seq=g*NS+s, t=b*TBLK+tt, hdim=m*128+p]
        hc = oc.transpose(0, 5, 1, 3, 4, 2).reshape(SEQS, KSIZE, H)
        out[c * ROWS_PER_CORE : (c + 1) * ROWS_PER_CORE] = hc.reshape(
            ROWS_PER_CORE, S, H
        )
    return out
